# revision 1
# baseline (speedup 1.0000x reference)
"""Self-contained Bass/Trainium2 kernel for the nn_Detector nms_detection problem.

kernel(**inputs) takes the FULL unsharded inputs (cls0..cls4, reg0..reg4,
locations) and returns (bbox [100,4] f32, classes [100] int32, scores [100] f32),
matching reference.reference().

Strategy (8-core SPMD, one Bass program):
  A: anchors sharded 8 ways (level-major); per-anchor max/argmax over the 80
     classes, sigmoid, threshold.  This is the memory-bound bulk (28 MB).
  AllGather of (score, class) blocks; phases below run replicated on all cores.
  B: exact global top-512 selection: per-partition top-24 value superset via
     max8/match_replace, 16-ary float bisection + bit-space endgame (probes are
     the next floats above lo via integer bit arithmetic on GPSIMD), exact
     boundary-tie resolution by smallest global position via masked min8.
     Top-512 is a provable truncation: per-level top-1000 quotas cannot cut it
     (512 <= 1000), and >= 100 of the top 512 survive NMS by a wide margin.
  C: selected positions extracted per partition (min8 rounds), compacted to a
     dense 512 via a PE one-hot matmul; score/class/box gathered by indirect
     DMA; viewport clip.
  D: 512x512 suppression matrix A[i,j] = (iou>0.5) & (key_i > key_j) and the
     order matrix ORD, both bf16, built with fused DVE ops against PE-broadcast
     row replicas.  Division-free iou test: inter*(1+th) > th*(area_i+area_j).
  E: 16 cluster-NMS iterations (15 scan steps + final recompute), each a
     keep-vector matvec against A on the PE with double-buffered keep.
  F: kept-rank matvec against ORD, top-100 scatter by rank (indirect DMA),
     coordinate transform, outputs.
"""
import numpy as np
import bass_rust
import concourse.bass as bass
import concourse.mybir as mybir
from concourse.bass_types import AP
from concourse.tile import TileContext

A_ = mybir.AluOpType
F32 = mybir.dt.float32
BF16 = mybir.dt.bfloat16
U32 = mybir.dt.uint32
I32 = mybir.dt.int32
AF = mybir.ActivationFunctionType

N_CORES = 8
NUM_CLASS = 80
N_TOTAL = 87296
SHARD = N_TOTAL // N_CORES
ACOLS = 86
PSHARD = 128 * ACOLS
N_PAD = N_CORES * PSHARD
FCOLS = N_PAD // 128
NMS_TH = 0.05
NMS_IOU = 0.5
NMS_ITERS = 15
R = 512
RCH = R // 128
NUMDETS = 100
TSUP = 3
SUPC = 8 * TSUP
BIG = 8388608.0  # 2^23: (BIG - g) exact for g < 2^17

_wsplit_ctr = [0]


def _split_multi_waits(nc):
    """This walrus accepts at most ONE semaphore wait per instruction; hoist
    extra waits onto same-engine NoOps inserted right before the instruction."""
    for f in nc.m.functions:
        for bb in f.blocks:
            new = []
            changed = False
            for inst in bb.instructions:
                si = inst.sync_info
                waits = list(si.on_wait) if si and si.on_wait else []
                if len(waits) > 1:
                    changed = True
                    for w in waits[:-1]:
                        _wsplit_ctr[0] += 1
                        nop = bass_rust.InstNoOp(
                            name=f"I-WSPLIT{_wsplit_ctr[0]}", ins=[], outs=[])
                        nop.engine = inst.engine
                        nop.sync_info = bass_rust.SyncInfo(on_wait=[w], on_update=[])
                        new.append(nop)
                    inst.sync_info = bass_rust.SyncInfo(
                        on_wait=[waits[-1]],
                        on_update=list(si.on_update) if si.on_update else [])
                new.append(inst)
            if changed:
                bb.instructions = new


def _b(ap, aplist):
    """Manual access pattern (for step-0 broadcasts)."""
    return AP(ap.tensor, ap.offset, aplist)


def build():
    nc = bass.Bass(num_devices=N_CORES)
    clsblk = nc.declare_dram_parameter("clsblk", [PSHARD, NUM_CLASS], F32, isOutput=False)
    regpad = nc.declare_dram_parameter("regpad", [N_PAD, 4], F32, isOutput=False)
    locations = nc.declare_dram_parameter("locations", [1, 6], F32, isOutput=False)
    argiota = nc.declare_dram_parameter("argiota", [128, ACOLS * NUM_CLASS], F32, isOutput=False)
    iota512 = nc.declare_dram_parameter("iota512", [1, R], F32, isOutput=False)
    lmat = nc.declare_dram_parameter("lmat", [128, 128], F32, isOutput=False)

    bbox_o = nc.declare_dram_parameter("bbox_o", [NUMDETS, 4], F32, isOutput=True)
    cls_o = nc.declare_dram_parameter("cls_o", [NUMDETS, 1], I32, isOutput=True)
    score_o = nc.declare_dram_parameter("score_o", [NUMDETS, 1], F32, isOutput=True)

    pspi = nc.dram_tensor("pspi", [2 * PSHARD], F32)
    ag_s = nc.dram_tensor("ag_s", [N_CORES, PSHARD], F32)
    ag_p = nc.dram_tensor("ag_p", [N_CORES, PSHARD], F32)
    dense_d = nc.dram_tensor("dense_d", [1, R], F32)
    rowrep_d = nc.dram_tensor("rowrep_d", [7, R], F32)
    outdense = nc.dram_tensor("outdense", [NUMDETS + 1, 6], F32)

    with TileContext(nc) as tc:
        with tc.tile_pool(name="sb", bufs=1) as sb, \
             tc.tile_pool(name="mjp", bufs=2) as mjp, \
             tc.tile_pool(name="ps", bufs=1, space="PSUM") as ps, \
             tc.tile_pool(name="ps2", bufs=2, space="PSUM") as ps2:

            # ---------------- Phase A ----------------
            s_ct = sb.tile([128, ACOLS], F32)
            pi_t = sb.tile([128, ACOLS], F32)
            chunk_cols = [22, 22, 21, 21]
            argio = sb.tile([128, ACOLS * NUM_CLASS], F32, tag="argio")
            nc.sync.dma_start(argio[:], argiota[:])
            a0 = 0
            for ac in chunk_cols:
                clst = sb.tile([128, ac * NUM_CLASS], F32, tag="clst")
                nc.sync.dma_start(
                    clst[:],
                    clsblk[:].rearrange("(p a) k -> p (a k)", p=128)[:, a0 * NUM_CLASS:(a0 + ac) * NUM_CLASS],
                )
                c3 = clst[:].rearrange("p (a k) -> p a k", k=NUM_CLASS)
                mv = sb.tile([128, ac], F32, tag="mv")
                nc.vector.tensor_reduce(mv[:], c3, axis=mybir.AxisListType.X, op=A_.max)
                t1 = sb.tile([128, ac * NUM_CLASS], F32, tag="t1")
                nc.vector.tensor_tensor(
                    out=t1[:].rearrange("p (a k) -> p a k", k=NUM_CLASS),
                    in0=c3,
                    in1=mv[:].to_broadcast([128, ac, NUM_CLASS]),
                    op=A_.is_equal,
                )
                nc.vector.scalar_tensor_tensor(
                    out=t1[:], in0=t1[:], scalar=-10000.0,
                    in1=argio[:, a0 * NUM_CLASS:(a0 + ac) * NUM_CLASS],
                    op0=A_.mult, op1=A_.add,
                )
                am = sb.tile([128, ac], F32, tag="am")
                nc.vector.tensor_reduce(
                    am[:], t1[:].rearrange("p (a k) -> p a k", k=NUM_CLASS),
                    axis=mybir.AxisListType.X, op=A_.min)
                sg = sb.tile([128, ac], F32, tag="sg")
                nc.scalar.activation(sg[:], mv[:], AF.Sigmoid)
                vl = sb.tile([128, ac], F32, tag="vl")
                nc.vector.tensor_scalar(vl[:], sg[:], float(NMS_TH), None, op0=A_.is_gt)
                nc.vector.tensor_tensor(out=s_ct[:, a0:a0 + ac], in0=sg[:], in1=vl[:], op=A_.mult)
                nc.vector.tensor_scalar(pi_t[:, a0:a0 + ac], am[:], 1.0, None, op0=A_.add)
                a0 += ac
            nc.sync.dma_start(pspi[0:PSHARD].rearrange("(p x) -> p x", p=128), s_ct[:])
            nc.sync.dma_start(pspi[PSHARD:2 * PSHARD].rearrange("(p x) -> p x", p=128), pi_t[:])

            # ---------------- AllGather ----------------
            nc.gpsimd.collective_compute(
                "AllGather", A_.bypass,
                replica_groups=[list(range(N_CORES))],
                ins=[pspi[0:PSHARD].rearrange("(a b) -> a b", b=128)],
                outs=[ag_s[:].rearrange("c (a b) -> (c a) b", b=128)],
            )
            nc.gpsimd.collective_compute(
                "AllGather", A_.bypass,
                replica_groups=[list(range(N_CORES))],
                ins=[pspi[PSHARD:2 * PSHARD].rearrange("(a b) -> a b", b=128)],
                outs=[ag_p[:].rearrange("c (a b) -> (c a) b", b=128)],
            )

            ssc = sb.tile([128, FCOLS], F32)
            nc.sync.dma_start(ssc[:], ag_s[:].rearrange("c x -> (c x)").rearrange("(p f) -> p f", p=128))

            # ---------------- Phase B ----------------
            gposi = sb.tile([128, FCOLS], I32)
            nc.gpsimd.iota(gposi[:], pattern=[[1, FCOLS]], base=0, channel_multiplier=FCOLS)
            gposf = sb.tile([128, FCOLS], F32)
            nc.vector.tensor_copy(gposf[:], gposi[:])

            ones1 = sb.tile([1, 128], F32)
            nc.vector.memset(ones1[:], 1.0)
            ones128 = sb.tile([128, 1], F32)
            nc.vector.memset(ones128[:], 1.0)
            io512 = sb.tile([1, R], F32)
            nc.sync.dma_start(io512[:], iota512[:])

            work = sb.tile([128, FCOLS], F32)
            nc.vector.tensor_copy(work[:], ssc[:])
            sup_s = sb.tile([128, SUPC], F32)
            for t in range(TSUP):
                nc.vector.max(out=sup_s[:, 8 * t:8 * t + 8], in_=work[:])
                if t < TSUP - 1:
                    nc.vector.match_replace(
                        out=work[:], in_to_replace=sup_s[:, 8 * t:8 * t + 8],
                        in_values=work[:], imm_value=-1.0)

            def part_reduce_scalar(src128, out11, tag):
                pstot = ps.tile([1, 1], F32, space="PSUM", tag="pssmall")
                nc.tensor.matmul(pstot[:], lhsT=ones128[:], rhs=src128, start=True, stop=True)
                nc.vector.tensor_copy(out11, pstot[:])

            def bcast_to_parts(src1x, out128x, tag):
                n = out128x.shape[1]
                pb = ps.tile([128, n], F32, space="PSUM", tag="psbig")
                nc.tensor.matmul(pb[:], lhsT=ones1[:], rhs=src1x, start=True, stop=True)
                nc.vector.tensor_copy(out128x, pb[:])

            NRND = 6
            lo = sb.tile([1, 1], F32)
            nc.vector.memset(lo[:], 0.0)
            probes = sb.tile([1, 15], F32)
            probes128 = sb.tile([128, 15], F32)
            tmat = sb.tile([128, 15 * SUPC], F32)
            percnt = sb.tile([128, 15], F32)
            cnt15 = sb.tile([1, 15], F32)
            bsel = sb.tile([1, 15], F32)
            bnum = sb.tile([1, 1], F32)

            def count_probes():
                nc.vector.tensor_tensor(
                    out=tmat[:].rearrange("p (k c) -> p k c", c=SUPC),
                    in0=_b(sup_s[:], [[SUPC, 128], [0, 15], [1, SUPC]]),
                    in1=_b(probes128[:], [[15, 128], [1, 15], [0, SUPC]]),
                    op=A_.is_gt)
                nc.vector.tensor_reduce(
                    percnt[:], tmat[:].rearrange("p (k c) -> p k c", c=SUPC),
                    axis=mybir.AxisListType.X, op=A_.add)
                pc15 = ps.tile([1, 15], F32, space="PSUM", tag="pssmall")
                nc.tensor.matmul(pc15[:], lhsT=ones128[:], rhs=percnt[:], start=True, stop=True)
                nc.vector.tensor_copy(cnt15[:], pc15[:])
                nc.vector.tensor_scalar(bsel[:], cnt15[:], R - 0.5, None, op0=A_.is_ge)
                nc.vector.tensor_reduce(bnum[:], bsel[:], axis=mybir.AxisListType.X, op=A_.add)

            delta = 1.0
            for rnd in range(NRND):
                dlt = delta / 16.0
                if rnd == 0:
                    nc.vector.tensor_scalar(probes[:], io512[:, 1:16], float(dlt), None, op0=A_.mult)
                else:
                    nc.vector.scalar_tensor_tensor(
                        out=probes[:], in0=io512[:, 1:16], scalar=float(dlt),
                        in1=lo[:].to_broadcast([1, 15]), op0=A_.mult, op1=A_.add)
                bcast_to_parts(probes[:], probes128[:], "prb")
                count_probes()
                nc.vector.scalar_tensor_tensor(
                    out=lo[:], in0=bnum[:], scalar=float(dlt),
                    in1=lo[:], op0=A_.mult, op1=A_.add)
                delta = dlt

            # bit-space endgame (exact): probes = next 15 floats above lo
            iotaI = sb.tile([1, 15], I32)
            nc.vector.tensor_copy(iotaI[:], io512[:, 1:16])
            bnumI = sb.tile([1, 1], I32)
            probesI = sb.tile([1, 15], I32)
            for brnd in range(3):
                loi = lo[:].bitcast(I32)
                nc.gpsimd.tensor_tensor(
                    out=probesI[:], in0=iotaI[:],
                    in1=_b(loi, [[1, 1], [0, 15]]), op=A_.add)
                bcast_to_parts(probesI[:].bitcast(F32), probes128[:], "prb")
                count_probes()
                nc.vector.tensor_copy(bnumI[:], bnum[:])
                nc.gpsimd.tensor_tensor(out=lo[:].bitcast(I32), in0=loi, in1=bnumI[:], op=A_.add)
            hi = sb.tile([1, 1], F32)
            nc.gpsimd.tensor_tensor(out=hi[:].bitcast(I32), in0=lo[:].bitcast(I32),
                                    in1=iotaI[:, 0:1], op=A_.add)

            hi128 = sb.tile([128, 1], F32)
            lo128 = sb.tile([128, 1], F32)
            bcast_to_parts(hi[:], hi128[:], "hi")
            bcast_to_parts(lo[:], lo128[:], "lo")
            tgt = sb.tile([128, SUPC], F32)
            pgt = sb.tile([128, 1], F32)
            nc.vector.tensor_tensor(out=tgt[:], in0=sup_s[:],
                                    in1=hi128[:].to_broadcast([128, SUPC]), op=A_.is_gt)
            nc.vector.tensor_reduce(pgt[:], tgt[:], axis=mybir.AxisListType.X, op=A_.add)
            cgt = sb.tile([1, 1], F32)
            part_reduce_scalar(pgt[:], cgt[:], "cgt")
            tie_need = sb.tile([1, 1], F32)
            nc.vector.tensor_scalar(tie_need[:], cgt[:], float(R), -1.0, op0=A_.subtract, op1=A_.mult)

            tm1 = sb.tile([128, FCOLS], F32)
            nc.vector.tensor_tensor(out=tm1[:], in0=ssc[:],
                                    in1=lo128[:].to_broadcast([128, FCOLS]), op=A_.is_gt)
            tm2 = sb.tile([128, FCOLS], F32)
            nc.vector.tensor_tensor(out=tm2[:], in0=ssc[:],
                                    in1=hi128[:].to_broadcast([128, FCOLS]), op=A_.is_le)
            nc.vector.tensor_tensor(out=tm1[:], in0=tm1[:], in1=tm2[:], op=A_.mult)
            nc.vector.tensor_scalar(tm2[:], gposf[:], float(BIG), -1.0, op0=A_.subtract, op1=A_.mult)
            nc.vector.tensor_tensor(out=tm2[:], in0=tm1[:], in1=tm2[:], op=A_.mult)
            nc.vector.tensor_scalar(tm2[:], tm2[:], float(BIG), None, op0=A_.subtract)
            tie8 = sb.tile([128, 8], F32)
            nc.vector.max(out=tie8[:], in_=tm2[:])
            tierow = sb.tile([1, 1024], F32)
            nc.sync.dma_start(tierow[:].rearrange("o (p c) -> o p c", p=128), tie8[:])
            gt8 = sb.tile([1, 8], F32)
            nc.vector.max(out=gt8[:], in_=tierow[:])
            nc.vector.tensor_scalar(gt8[:], gt8[:], -1.0, None, op0=A_.mult)
            tn1 = sb.tile([1, 1], F32)
            nc.vector.tensor_scalar(tn1[:], tie_need[:], 1.0, None, op0=A_.subtract)
            oh8 = sb.tile([1, 8], F32)
            nc.vector.tensor_tensor(out=oh8[:], in0=io512[:, 0:8],
                                    in1=tn1[:].to_broadcast([1, 8]), op=A_.is_equal)
            nc.vector.tensor_tensor(out=oh8[:], in0=oh8[:], in1=gt8[:], op=A_.mult)
            mstar = sb.tile([1, 1], F32)
            nc.vector.tensor_reduce(mstar[:], oh8[:], axis=mybir.AxisListType.X, op=A_.add)
            mstar128 = sb.tile([128, 1], F32)
            bcast_to_parts(mstar[:], mstar128[:], "ms")

            self_t = sb.tile([128, FCOLS], F32)
            nc.vector.tensor_tensor(out=self_t[:], in0=ssc[:],
                                    in1=hi128[:].to_broadcast([128, FCOLS]), op=A_.is_gt)
            selt2 = sb.tile([128, FCOLS], F32)
            nc.vector.tensor_tensor(out=selt2[:], in0=gposf[:],
                                    in1=mstar128[:].to_broadcast([128, FCOLS]), op=A_.is_le)
            nc.vector.tensor_tensor(out=selt2[:], in0=selt2[:], in1=tm1[:], op=A_.mult)
            nc.vector.tensor_tensor(out=self_t[:], in0=self_t[:], in1=selt2[:], op=A_.max)

            # ---------------- Phase C ----------------
            nc.vector.tensor_scalar(selt2[:], gposf[:], float(BIG), -1.0, op0=A_.subtract, op1=A_.mult)
            nc.vector.tensor_tensor(out=selt2[:], in0=self_t[:], in1=selt2[:], op=A_.mult)
            nc.vector.tensor_scalar(selt2[:], selt2[:], float(BIG), None, op0=A_.subtract)
            sup_ng = sb.tile([128, SUPC], F32)
            for t in range(TSUP):
                nc.vector.max(out=sup_ng[:, 8 * t:8 * t + 8], in_=selt2[:])
                if t < TSUP - 1:
                    nc.vector.match_replace(
                        out=selt2[:], in_to_replace=sup_ng[:, 8 * t:8 * t + 8],
                        in_values=selt2[:], imm_value=-float(BIG))
            sup_g = sb.tile([128, SUPC], F32)
            nc.vector.tensor_scalar(sup_g[:], sup_ng[:], -1.0, None, op0=A_.mult)
            sel24 = sb.tile([128, SUPC], F32)
            nc.vector.tensor_scalar(sel24[:], sup_g[:], 1.0e6, None, op0=A_.is_lt)

            onesS = sb.tile([128, SUPC], F32)
            nc.vector.memset(onesS[:], 1.0)
            incl = sb.tile([128, SUPC], F32)
            nc.vector.tensor_tensor_scan(
                out=incl[:], data0=sel24[:], data1=onesS[:], initial=0.0,
                op0=A_.add, op1=A_.mult)
            excl = sb.tile([128, SUPC], F32)
            nc.vector.tensor_tensor(out=excl[:], in0=incl[:], in1=sel24[:], op=A_.subtract)
            ptot = sb.tile([128, 1], F32)
            nc.vector.tensor_copy(ptot[:], incl[:, SUPC - 1:SUPC])
            lmat_t = sb.tile([128, 128], F32)
            nc.sync.dma_start(lmat_t[:], lmat[:])
            poff_ps = ps.tile([128, 1], F32, space="PSUM", tag="pssmall")
            nc.tensor.matmul(poff_ps[:], lhsT=lmat_t[:], rhs=ptot[:], start=True, stop=True)
            poff = sb.tile([128, 1], F32)
            nc.vector.tensor_copy(poff[:], poff_ps[:])
            offs = sb.tile([128, SUPC], F32)
            nc.vector.tensor_tensor(out=offs[:], in0=excl[:],
                                    in1=poff[:].to_broadcast([128, SUPC]), op=A_.add)
            nc.vector.tensor_scalar(excl[:], offs[:], float(R), None, op0=A_.subtract)
            nc.vector.tensor_tensor(out=excl[:], in0=excl[:], in1=sel24[:], op=A_.mult)
            nc.vector.tensor_scalar(offs[:], excl[:], float(R), None, op0=A_.add)

            io512B = sb.tile([128, R], F32)
            bcast_to_parts(io512[:, 0:R], io512B[:], "io512")
            psg = ps.tile([1, R], F32, space="PSUM", tag="dens")
            for j in range(SUPC):
                mj = mjp.tile([128, R], F32, tag="mj")
                nc.vector.tensor_scalar(mj[:], io512B[:], offs[:, j:j + 1], None, op0=A_.is_equal)
                nc.tensor.matmul(psg[:], lhsT=sup_g[:, j:j + 1], rhs=mj[:],
                                 start=(j == 0), stop=(j == SUPC - 1))
            densg_row = sb.tile([1, R], F32)
            nc.vector.tensor_copy(densg_row[:], psg[:])
            nc.sync.dma_start(dense_d[:], densg_row[:])

            dsoa_g = sb.tile([128, RCH], F32)
            nc.sync.dma_start(dsoa_g[:], dense_d[0, :].rearrange("(j p) -> p j", p=128))
            goff = sb.tile([128, RCH], U32)
            nc.vector.tensor_copy(goff[:], dsoa_g[:])
            ds_s = sb.tile([128, RCH], F32)
            dpi = sb.tile([128, RCH], F32)
            bxa = sb.tile([128, 4 * RCH], F32)
            for j in range(RCH):
                nc.gpsimd.indirect_dma_start(
                    out=ds_s[:, j:j + 1], out_offset=None,
                    in_=ag_s[:].rearrange("c x -> (c x) ()"),
                    in_offset=bass.IndirectOffsetOnAxis(ap=goff[:, j:j + 1], axis=0))
                nc.gpsimd.indirect_dma_start(
                    out=dpi[:, j:j + 1], out_offset=None,
                    in_=ag_p[:].rearrange("c x -> (c x) ()"),
                    in_offset=bass.IndirectOffsetOnAxis(ap=goff[:, j:j + 1], axis=0))
                nc.gpsimd.indirect_dma_start(
                    out=bxa[:, 4 * j:4 * j + 4], out_offset=None,
                    in_=regpad[:, :],
                    in_offset=bass.IndirectOffsetOnAxis(ap=goff[:, j:j + 1], axis=0))

            locT = sb.tile([1, 6], F32)
            nc.sync.dma_start(locT[:], locations[:])
            locB = sb.tile([128, 6], F32)
            bcast_to_parts(locT[:], locB[:], "loc")
            y1v = _b(bxa[:], [[4 * RCH, 128], [4, RCH]])
            x1v = _b(bxa[:, 1:], [[4 * RCH, 128], [4, RCH]])
            y2v = _b(bxa[:, 2:], [[4 * RCH, 128], [4, RCH]])
            x2v = _b(bxa[:, 3:], [[4 * RCH, 128], [4, RCH]])
            cly1 = sb.tile([128, RCH], F32)
            clx1 = sb.tile([128, RCH], F32)
            cly2 = sb.tile([128, RCH], F32)
            clx2 = sb.tile([128, RCH], F32)
            nc.vector.tensor_scalar(cly1[:], y1v, locB[:, 0:1], None, op0=A_.max)
            nc.vector.tensor_scalar(clx1[:], x1v, locB[:, 1:2], None, op0=A_.max)
            nc.vector.tensor_scalar(cly2[:], y2v, locB[:, 2:3], None, op0=A_.min)
            nc.vector.tensor_scalar(clx2[:], x2v, locB[:, 3:4], None, op0=A_.min)
            hh = sb.tile([128, RCH], F32)
            ww = sb.tile([128, RCH], F32)
            nc.vector.tensor_tensor(out=hh[:], in0=cly2[:], in1=cly1[:], op=A_.subtract)
            nc.vector.tensor_scalar(hh[:], hh[:], 0.0, None, op0=A_.max)
            nc.vector.tensor_tensor(out=ww[:], in0=clx2[:], in1=clx1[:], op=A_.subtract)
            nc.vector.tensor_scalar(ww[:], ww[:], 0.0, None, op0=A_.max)
            tha = sb.tile([128, RCH], F32)
            nc.vector.tensor_tensor(out=tha[:], in0=hh[:], in1=ww[:], op=A_.mult)
            nc.vector.tensor_scalar(tha[:], tha[:], float(NMS_IOU), None, op0=A_.mult)

            for q, tile_q in enumerate([cly1, clx1, cly2, clx2, tha, ds_s, dsoa_g]):
                nc.sync.dma_start(
                    rowrep_d[q, :].rearrange("(j p) -> p j", p=128), tile_q[:])
            rbs = []
            for q in range(7):
                rr = sb.tile([1, R], F32, tag=f"rr{q}")
                nc.sync.dma_start(rr[:], rowrep_d[q:q + 1, :])
                rb = sb.tile([128, R], F32, tag=f"rb{q}")
                bcast_to_parts(rr[:], rb[:], f"rb{q}")
                rbs.append(rb)
            RB_y1, RB_x1, RB_y2, RB_x2, RB_tha, RB_s, RB_g = rbs

            # ---------------- Phase D ----------------
            Abf = []
            Obf = []
            for ci in range(RCH):
                y1c = cly1[:, ci:ci + 1]
                x1c = clx1[:, ci:ci + 1]
                y2c = cly2[:, ci:ci + 1]
                x2c = clx2[:, ci:ci + 1]
                thac = tha[:, ci:ci + 1]
                sc = ds_s[:, ci:ci + 1]
                gc = dsoa_g[:, ci:ci + 1]
                m1 = sb.tile([128, R], F32, tag="bm1")
                nc.vector.tensor_scalar(m1[:], RB_y1[:], y1c, None, op0=A_.max)
                ih = sb.tile([128, R], F32, tag="bih")
                nc.vector.scalar_tensor_tensor(
                    out=ih[:], in0=RB_y2[:], scalar=y2c, in1=m1[:], op0=A_.min, op1=A_.subtract)
                nc.vector.tensor_scalar(m1[:], RB_x1[:], x1c, None, op0=A_.max)
                iw = sb.tile([128, R], F32, tag="biw")
                nc.vector.scalar_tensor_tensor(
                    out=iw[:], in0=RB_x2[:], scalar=x2c, in1=m1[:], op0=A_.min, op1=A_.subtract)
                c1 = sb.tile([128, R], F32, tag="bc1")
                nc.vector.tensor_tensor(out=c1[:], in0=ih[:], in1=iw[:], op=A_.min)
                nc.vector.tensor_tensor(out=ih[:], in0=ih[:], in1=iw[:], op=A_.mult)
                nc.vector.tensor_scalar(iw[:], RB_tha[:], thac, None, op0=A_.add)
                nc.vector.scalar_tensor_tensor(
                    out=ih[:], in0=ih[:], scalar=1.0 + float(NMS_IOU), in1=iw[:],
                    op0=A_.mult, op1=A_.subtract)
                nc.vector.tensor_tensor(out=c1[:], in0=c1[:], in1=ih[:], op=A_.min)
                o1 = sb.tile([128, R], F32, tag="bo1")
                nc.vector.tensor_scalar(o1[:], RB_s[:], sc, None, op0=A_.is_lt)
                oe = sb.tile([128, R], F32, tag="boe")
                nc.vector.tensor_scalar(oe[:], RB_s[:], sc, None, op0=A_.is_equal)
                o3 = sb.tile([128, R], F32, tag="bo3")
                nc.vector.tensor_scalar(o3[:], RB_g[:], gc, None, op0=A_.is_gt)
                nc.vector.tensor_tensor(out=oe[:], in0=oe[:], in1=o3[:], op=A_.mult)
                nc.vector.tensor_tensor(out=o1[:], in0=o1[:], in1=oe[:], op=A_.max)
                ab = sb.tile([128, R], BF16, tag=f"ab{ci}")
                nc.vector.scalar_tensor_tensor(
                    out=ab[:], in0=c1[:], scalar=0.0, in1=o1[:], op0=A_.is_gt, op1=A_.mult)
                ob = sb.tile([128, R], BF16, tag=f"ob{ci}")
                nc.vector.tensor_copy(ob[:], o1[:])
                Abf.append(ab)
                Obf.append(ob)

            # ---------------- Phase E ----------------
            kcur = sb.tile([128, RCH], BF16, tag="keepA")
            knxt = sb.tile([128, RCH], BF16, tag="keepB")
            nc.vector.memset(kcur[:], 1.0)
            for it in range(NMS_ITERS + 1):
                for cj in range(RCH):
                    pcnt = ps2.tile([128, 1], F32, space="PSUM", tag="psmv")
                    for ci in range(RCH):
                        nc.tensor.matmul(
                            pcnt[:], lhsT=Abf[ci][:, cj * 128:(cj + 1) * 128],
                            rhs=kcur[:, ci:ci + 1],
                            start=(ci == 0), stop=(ci == RCH - 1))
                    nc.vector.tensor_scalar(knxt[:, cj:cj + 1], pcnt[:], 0.5, None, op0=A_.is_lt)
                kcur, knxt = knxt, kcur
            keepb = kcur

            vld = sb.tile([128, RCH], BF16)
            nc.vector.tensor_scalar(vld[:], ds_s[:], float(NMS_TH), None, op0=A_.is_gt)
            kv = sb.tile([128, RCH], BF16)
            nc.vector.tensor_tensor(out=kv[:], in0=keepb[:], in1=vld[:], op=A_.mult)
            krank = sb.tile([128, RCH], F32)
            for cj in range(RCH):
                pr = ps2.tile([128, 1], F32, space="PSUM", tag="psmv")
                for ci in range(RCH):
                    nc.tensor.matmul(
                        pr[:], lhsT=Obf[ci][:, cj * 128:(cj + 1) * 128],
                        rhs=kv[:, ci:ci + 1],
                        start=(ci == 0), stop=(ci == RCH - 1))
                nc.vector.tensor_copy(krank[:, cj:cj + 1], pr[:])

            # ---------------- Phase F ----------------
            kv32 = sb.tile([128, RCH], F32)
            nc.vector.tensor_copy(kv32[:], kv[:])
            osel = sb.tile([128, RCH], F32)
            nc.vector.scalar_tensor_tensor(
                out=osel[:], in0=krank[:], scalar=NUMDETS - 0.5, in1=kv32[:],
                op0=A_.is_le, op1=A_.mult)
            offo = sb.tile([128, RCH], F32)
            nc.vector.tensor_scalar(offo[:], krank[:], float(NUMDETS), None, op0=A_.subtract)
            nc.vector.tensor_tensor(out=offo[:], in0=offo[:], in1=osel[:], op=A_.mult)
            nc.vector.tensor_scalar(offo[:], offo[:], float(NUMDETS), None, op0=A_.add)
            offu = sb.tile([128, RCH], U32)
            nc.vector.tensor_copy(offu[:], offo[:])

            dv = sb.tile([128, 1], F32, tag="dv")
            sy = sb.tile([128, 1], F32)
            sx = sb.tile([128, 1], F32)
            nc.vector.tensor_tensor(out=dv[:], in0=locB[:, 2:3], in1=locB[:, 0:1], op=A_.subtract)
            nc.vector.tensor_scalar(dv[:], dv[:], 1.0e-6, None, op0=A_.max)
            nc.vector.reciprocal(sy[:], dv[:])
            nc.vector.tensor_tensor(out=sy[:], in0=sy[:], in1=locB[:, 4:5], op=A_.mult)
            nc.vector.tensor_tensor(out=dv[:], in0=locB[:, 3:4], in1=locB[:, 1:2], op=A_.subtract)
            nc.vector.tensor_scalar(dv[:], dv[:], 1.0e-6, None, op0=A_.max)
            nc.vector.reciprocal(sx[:], dv[:])
            nc.vector.tensor_tensor(out=sx[:], in0=sx[:], in1=locB[:, 5:6], op=A_.mult)

            aos = sb.tile([128, 6 * RCH], F32)

            def aosf(q):
                return _b(aos[:, q:], [[6 * RCH, 128], [6, RCH]])

            for (src, vq, sc_ap, scale_ap) in [
                (cly1, 0, locB[:, 0:1], sy),
                (clx1, 1, locB[:, 1:2], sx),
                (cly2, 2, locB[:, 0:1], sy),
                (clx2, 3, locB[:, 1:2], sx),
            ]:
                tmp = sb.tile([128, RCH], F32, tag="ftmp")
                nc.vector.tensor_scalar(tmp[:], src[:], sc_ap, None, op0=A_.subtract)
                nc.vector.tensor_tensor(
                    out=aosf(vq), in0=tmp[:],
                    in1=scale_ap[:].to_broadcast([128, RCH]), op=A_.mult)
            nc.vector.tensor_copy(aosf(4), dpi[:])
            nc.vector.tensor_copy(aosf(5), ds_s[:])

            zer = sb.tile([NUMDETS + 1, 6], F32)
            nc.vector.memset(zer[:], 0.0)
            nc.sync.dma_start(outdense[:, :], zer[:])
            for j in range(RCH):
                nc.gpsimd.indirect_dma_start(
                    out=outdense[:, :],
                    out_offset=bass.IndirectOffsetOnAxis(ap=offu[:, j:j + 1], axis=0),
                    in_=aos[:, 6 * j:6 * j + 6],
                    in_offset=None)
            od = sb.tile([NUMDETS + 1, 6], F32)
            nc.sync.dma_start(od[:], outdense[:, :])
            nc.sync.dma_start(bbox_o[:, :], od[0:NUMDETS, 0:4])
            odi = sb.tile([NUMDETS, 1], I32)
            nc.vector.tensor_copy(odi[:], od[0:NUMDETS, 4:5])
            nc.sync.dma_start(cls_o[:, :], odi[:])
            nc.sync.dma_start(score_o[:, :], od[0:NUMDETS, 5:6])

    _split_multi_waits(nc)
    return nc


def host_inputs(inputs):
    cls_full = np.concatenate([np.asarray(inputs[f"cls{i}"]).reshape(-1, NUM_CLASS)
                               for i in range(5)], axis=0).astype(np.float32)
    reg_full = np.concatenate([np.asarray(inputs[f"reg{i}"]).reshape(-1, 4)
                               for i in range(5)], axis=0).astype(np.float32)
    loc = np.asarray(inputs["locations"], dtype=np.float32).reshape(1, 6)

    cls_pad = np.full((N_PAD, NUM_CLASS), -100.0, dtype=np.float32)
    reg_pad = np.zeros((N_PAD, 4), dtype=np.float32)
    for c in range(N_CORES):
        cls_pad[c * PSHARD: c * PSHARD + SHARD] = cls_full[c * SHARD:(c + 1) * SHARD]
        reg_pad[c * PSHARD: c * PSHARD + SHARD] = reg_full[c * SHARD:(c + 1) * SHARD]

    argiota = np.tile(np.arange(NUM_CLASS, dtype=np.float32) + 10000.0, ACOLS)
    argiota = np.broadcast_to(argiota[None, :], (128, ACOLS * NUM_CLASS)).copy()
    iota512 = np.arange(R, dtype=np.float32).reshape(1, R)
    lmat = (np.arange(128)[:, None] < np.arange(128)[None, :]).astype(np.float32)

    in_maps = []
    for c in range(N_CORES):
        in_maps.append({
            "clsblk": np.ascontiguousarray(cls_pad[c * PSHARD:(c + 1) * PSHARD]),
            "regpad": reg_pad,
            "locations": loc,
            "argiota": argiota,
            "iota512": iota512,
            "lmat": lmat,
        })
    return in_maps


_CACHE = {}


def kernel(**inputs):
    from concourse.bass_utils import run_bass_kernel_spmd
    if "nc" not in _CACHE:
        _CACHE["nc"] = build()
    nc = _CACHE["nc"]
    in_maps = host_inputs(inputs)
    res = run_bass_kernel_spmd(nc, in_maps, core_ids=list(range(N_CORES)))
    r0 = res.results[0]
    bbox = np.asarray(r0["bbox_o"], dtype=np.float32)
    cls = np.asarray(r0["cls_o"], dtype=np.int32).reshape(NUMDETS)
    scores = np.asarray(r0["score_o"], dtype=np.float32).reshape(NUMDETS)
    return bbox, cls, scores
